# revision 4
# baseline (speedup 1.0000x reference)
"""Bass/Trainium2 kernel for nn_ADJ_FirstLayer (gnn_message_passing).

reference(x):  N = x.shape[0]; M = N + 4
  A = eye(M); A[N:, N:] = 1  (symmetric)
  d = rowsum(A)^-0.5  ->  d[i] = 1 for i < N, 0.5 for i >= N
  out = d[:,None] * A.T * d[None,:]
  => out = identity on first N diagonal entries, bottom-right 4x4 block = 0.25

The output depends only on N, not on x's values, and is 99.99% zeros:
a pure HBM-write-bandwidth problem (M*M*4 bytes = 268.7 MB).

Sharding: row-shard the (M x M) output across 8 cores, R = 1025 rows each
(8*1025 = 8200 >= 8196; the host trims the last 4 garbage rows). Each core
zero-fills its (R, M) block with large SBUF->DRAM DMAs, then writes its
piece of the diagonal with a dynamic-offset strided DMA (stride M+1). All
rank-dependence (diagonal column offset, diagonal values, 4x4 corner
values) is carried in tiny per-core input arrays; the SPMD program is
identical on every core. x itself never touches the device - it does not
appear in the math.

Latency tricks (measured on HW):
 - Everything is issued from the Sync sequencer (HWDGE), which starts
   executing ~0.1 us into the NEFF; the compute engines only wake up
   ~7.5 us in, so no memset: the SBUF zero span is loaded by DMA from a
   tiny host-provided zero input, and each 2 MB chunk DMA re-reads it via
   a stride-0 (broadcast) source AP. The zero stream starts ~2.5 us in.
 - The 1024-element main diagonal segment reads its values from a
   [128, 8] SBUF tile so the HWDGE spreads the 1024 single-element
   descriptors across all 16 SDMA engines (a [1, 1024] source pins them
   all to engine 0: ~11 us serial tail).
 - Row 1024's diagonal element is written separately (1-element DMA at a
   second dynamic offset); on core 7 that write is aimed at a trimmed
   garbage row.
"""
import sys

if "/opt/trn_rl_repo" not in sys.path:
    sys.path.insert(0, "/opt/trn_rl_repo")

import numpy as np

import concourse.bass as bass
from concourse import mybir
from concourse.bass_utils import run_bass_kernel_spmd

N = 8192
M = N + 4            # 8196
N_CORES = 8
R = 1025             # rows per core; 8*1025 = 8200, host trims to 8196
FLAT = R * M         # 8,400,900 elements per core

DIAG_MAIN = 1024     # main diagonal segment length = 128 partitions * 8
BLK_ROW0 = 1017      # local row of the 4x4 ones block on core 7

SPAN = 512           # zero span elements per partition (2 KB descriptors)
REP = 8              # broadcast repeats per chunk
ZT = SPAN * REP      # 4096: chunk free-dim elements per partition
CHUNK = 128 * ZT     # 524,288 elements (2 MB) per chunk DMA
N_CHUNKS = FLAT // CHUNK          # 16 full chunks
REM = FLAT - N_CHUNKS * CHUNK     # 12,292 = 12*1024 + 4

C1_MAX = (N_CORES - 2) * R + DIAG_MAIN * (M + 1)  # largest tail offset (core 6)

_nc_cache = None


def _build():
    nc = bass.Bass()
    zsrc = nc.declare_dram_parameter("zsrc", [128, SPAN], mybir.dt.float32, isOutput=False)
    dvals = nc.declare_dram_parameter("dvals", [128, 8], mybir.dt.float32, isOutput=False)
    svals = nc.declare_dram_parameter("svals", [1, 24], mybir.dt.float32, isOutput=False)
    offs = nc.declare_dram_parameter("offs", [1, 2], mybir.dt.int32, isOutput=False)
    out = nc.declare_dram_parameter("out", [R, M], mybir.dt.float32, isOutput=True)
    out_flat = out[:].flatten()

    with (
        nc.Block() as block,
        nc.semaphore("zt_sem") as zt_sem,
        nc.semaphore("in_sem") as in_sem,
        nc.semaphore("zdma_sem") as zdma_sem,
        nc.semaphore("fdma_sem") as fdma_sem,
        nc.sbuf_tensor("ztile", [128, SPAN], mybir.dt.float32) as ztile,
        nc.sbuf_tensor("dtile", [128, 8], mybir.dt.float32) as dtile,
        nc.sbuf_tensor("stile", [1, 24], mybir.dt.float32) as stile,
        nc.sbuf_tensor("otile", [1, 2], mybir.dt.int32) as otile,
        nc.sync.register() as r0,
        nc.sync.register() as r1,
    ):
        @block.sync
        def _(sync):
            sync.dma_start(out=ztile[:, :], in_=zsrc[:, :]).then_inc(zt_sem, 16)
            sync.dma_start(out=dtile[:, :], in_=dvals[:, :]).then_inc(in_sem, 16)
            sync.dma_start(out=stile[:, :], in_=svals[:, :]).then_inc(in_sem, 16)
            sync.dma_start(out=otile[:, :], in_=offs[:, :]).then_inc(in_sem, 16)
            sync.wait_ge(zt_sem, 16)

            zap = ztile[:, :]
            pstride = zap.ap[0][0]
            # zero-fill the whole (R, M) block: 16 x 2MB broadcast-source DMAs
            zsem = 0
            for k in range(N_CHUNKS):
                dst = bass.AP(out_flat.tensor, k * CHUNK,
                              [[ZT, 128], [SPAN, REP], [1, SPAN]])
                src = bass.AP(zap.tensor, zap.offset,
                              [[pstride, 128], [0, REP], [1, SPAN]])
                sync.dma_start(out=dst, in_=src).then_inc(zdma_sem, 16)
                zsem += 16
            # remainder: 12,292 = 12*1024 + 4 elements
            dst = bass.AP(out_flat.tensor, N_CHUNKS * CHUNK, [[1024, 12], [1, 1024]])
            src = bass.AP(zap.tensor, zap.offset, [[pstride, 2], [0, 12], [1, 512]])
            sync.dma_start(out=dst, in_=src).then_inc(zdma_sem, 16)
            zsem += 16
            dst = bass.AP(out_flat.tensor, N_CHUNKS * CHUNK + 12288, [[1, 4]])
            sync.dma_start(out=dst, in_=ztile[0:1, 0:4]).then_inc(zdma_sem, 16)
            zsem += 16

            # load diagonal offsets while the zero stream runs
            sync.wait_ge(in_sem, 48)
            sync.reg_load(r0, otile[0:1, 0:1])
            sync.reg_load(r1, otile[0:1, 1:2])
            c0 = sync.snap(r0)
            c1 = sync.snap(r1)
            d0 = out_flat[0 : (N_CORES - 1) * R + 1][bass.ds(c0, 1)].offset
            d1 = out_flat[0 : C1_MAX + 1][bass.ds(c1, 1)].offset
            main_ap = bass.AP(out_flat.tensor, d0, [[M + 1, DIAG_MAIN]])
            tail_ap = bass.AP(out_flat.tensor, d1, [[1, 1]])

            sync.wait_ge(zdma_sem, zsem)
            # diagonal writes at dynamic offsets (stride M+1 walks the diagonal)
            with nc.allow_non_contiguous_dma(reason="diagonal scatter"):
                sync.dma_start(out=main_ap, in_=dtile[:, :]).then_inc(fdma_sem, 16)
                sync.dma_start(out=tail_ap, in_=stile[0:1, 0:1]).then_inc(fdma_sem, 16)
            # 4x4 corner block (values 0.0 on cores 0-6, 0.25 on core 7)
            sync.dma_start(
                out=out[BLK_ROW0 : BLK_ROW0 + 4, N : N + 4],
                in_=stile[0:1, 4:20],
            ).then_inc(fdma_sem, 16)
            sync.wait_ge(fdma_sem, 48)
    return nc


def _in_maps():
    maps = []
    zsrc = np.zeros((128, SPAN), np.float32)
    for r in range(N_CORES):
        dvals = np.ones((128, 8), np.float32)
        svals = np.zeros((1, 24), np.float32)
        offs = np.zeros((1, 2), np.int32)
        c0 = r * R
        if r < N_CORES - 1:
            svals[0, 0] = 1.0                  # row-1024 diagonal element
            c1 = c0 + DIAG_MAIN * (M + 1)
        else:
            # core 7: global rows 7175..8199; 8192..8195 hold the ones-block,
            # 8196..8199 are trimmed garbage.
            flat = dvals.reshape(-1)
            flat[BLK_ROW0 : BLK_ROW0 + 4] = 0.25   # diag entries in the 4x4 block
            flat[BLK_ROW0 + 4 :] = 0.0             # rows 8196+: garbage, any value
            svals[0, 0] = 0.0
            svals[0, 4:20] = 0.25              # the 4x4 ones block * 0.25
            c1 = (DIAG_MAIN - 3) * M           # inside garbage row 1021
        offs[0, 0] = c0
        offs[0, 1] = c1
        maps.append({"zsrc": zsrc, "dvals": dvals, "svals": svals, "offs": offs})
    return maps


def _run(trace=False, **kwargs):
    global _nc_cache
    if _nc_cache is None:
        _nc_cache = _build()
    return run_bass_kernel_spmd(
        _nc_cache, _in_maps(), core_ids=list(range(N_CORES)), trace=trace, **kwargs
    )


def kernel(x: np.ndarray) -> np.ndarray:
    assert x.shape == (N, 2048), x.shape
    res = _run()
    blocks = [res.results[r]["out"] for r in range(N_CORES)]
    return np.concatenate(blocks, axis=0)[:M]


if __name__ == "__main__":
    out = kernel(np.zeros((N, 2048), np.float32))
    print(out.shape, out.dtype)
